# revision 26
# baseline (speedup 1.0000x reference)
"""Trainium2 Bass kernel for nn_Conv_27693949125154.

Each 128-dim vector is a 16x8 image; valid 3x3 conv with the fixed kernel
[[1,0,1],[0,1,0],[1,0,1]] then relu -> 84 outputs (14x6).

Stencil decomposition in FLAT pixel space (letter layout 16x8, so taps at
flat offsets +2 / +16 stay inside the letter for every output we keep):
    h[n] = x[n] + x[n+2]        (horizontal pair)
    v[n] = h[n] + h[n+16]       (adds the two rows below)
    out(i,j) = relu(v[8i+j] + x[8i+j+9])   (center tap, i<14, j<6)

Engine strategy (measured): fp32 tensor_tensor on DVE is capped at
1 elem/cycle; bf16 with contiguous step-1 4B-aligned operands runs 2x
(2x_1P mode). So inputs are cast f32->bf16 INSIDE the load DMA (SWDGE on
gpsimd; HBM traffic unchanged), h and v are flat fully-aligned bf16 adds
at 2x, and only the center-tap add (+9 is odd -> 18B offset, misaligned)
runs at 1x over the compact 84/letter outputs. GpSimd must NOT run
elementwise ops concurrently with DVE (shared SBUF port pair - measured
2.5x mutual slowdown), it only issues the SWDGE load descriptors.
ACT does relu + bf16->f32 cast in one op; stores ride the scalar HWDGE
ring, fully overlapped with the loads.

Pure data parallel over 8 NeuronCores (batch sharding, no comm).
"""

import numpy as np

import concourse.bass as bass
import concourse.mybir as mybir
from concourse import tile
from concourse.bass_utils import run_bass_kernel_spmd

# Full problem: x (16384, 14, 128) f32 -> out (16384, 14, 84) f32
B, W, L = 16384, 14, 128
OUT = 84
N_CORES = 8
ROWS = B * W                     # 229376 letters total
ROWS_PER_CORE = ROWS // N_CORES  # 28672
P = 128                          # SBUF partitions

F32 = mybir.dt.float32
BF16 = mybir.dt.bfloat16


def split_multi_waits(nc, max_waits=1):
    """walrus CoreV3 codegen rejects instructions with several sync-wait
    conditions; hoist extras onto NOPs inserted just before, same engine."""
    for f in nc.m.functions:
        for blk in f.blocks:
            new = []
            for inst in blk.instructions:
                si = inst.sync_info
                if si is not None and si.on_wait and len(si.on_wait) > max_waits:
                    waits = list(si.on_wait)
                    head, tail = waits[:-max_waits], waits[-max_waits:]
                    for k, w in enumerate(head):
                        new.append(
                            mybir.InstNoOp(
                                name=f"{inst.name}-wsplit{k}",
                                engine=inst.engine,
                                ins=[],
                                outs=[],
                                sync_info=mybir.SyncInfo(on_wait=[w], on_update=[]),
                            )
                        )
                    inst.sync_info = mybir.SyncInfo(
                        on_wait=tail, on_update=list(si.on_update)
                    )
                new.append(inst)
            blk.instructions = new


def build_program(rows=ROWS_PER_CORE, read_sizes=None, chunk_sizes=None,
                  split_waits=True, s_bufs=None, h_bufs=2, v_bufs=2,
                  o_bufs=6, store_delay=3, num_devices=1):
    """Per-core program: x [rows,128] f32 -> y [rows,84] f32."""
    t_total = rows // P                  # letters per partition (224)
    if chunk_sizes is None:
        chunk_sizes = [4, 10, 14, 28, 28, 28, 28, 28, 28, 21, 7]
    if read_sizes is None:
        # one slab per compute chunk: no chunk ever waits for letters
        # past its own end, and the first compute starts ASAP
        read_sizes = list(chunk_sizes)
    if s_bufs is None:
        s_bufs = len(chunk_sizes)
    assert sum(read_sizes) == t_total and sum(chunk_sizes) == t_total
    t_c_max = max(chunk_sizes)

    nc = bass.Bass(
        "TRN2", target_bir_lowering=False, debug=False, num_devices=num_devices
    )
    x = nc.dram_tensor("x", [rows, L], F32, kind="ExternalInput")
    y = nc.dram_tensor("y", [rows, OUT], F32, kind="ExternalOutput")


    # partition p holds letters [p*t_total, (p+1)*t_total)
    xf = x.ap().rearrange("(p t) m -> p (t m)", p=P)   # [P, t_total*128] f32
    yf = y.ap().rearrange("(p t) m -> p (t m)", p=P)   # [P, t_total*84] f32

    with tile.TileContext(nc) as tc:
        with (
            tc.tile_pool(name="xin", bufs=1) as xin_pool,
            tc.tile_pool(name="hpool", bufs=h_bufs) as hpool,
            tc.tile_pool(name="spool", bufs=s_bufs) as spool,
        ):
            xt = xin_pool.tile([P, t_total * L], BF16, tag="x")
            # all reads upfront (deep read-ahead), f32->bf16 cast in the DMA
            off = 0
            for sz in read_sizes:
                nc.gpsimd.dma_start(
                    out=xt[:, off * L : (off + sz) * L],
                    in_=xf[:, off * L : (off + sz) * L],
                )
                off += sz

            X4 = xt.rearrange("p (t i j) -> p t i j", i=16, j=8)  # [P,t,16,8]
            pending = []   # stores issue on gpsimd AFTER all reads: SWDGE
            off = 0        # shares one queue, so the FIFO gives reads
            for t_c in chunk_sizes:  # strict HBM priority over stores.
                N = t_c * L
                c0 = off * L
                # h[n] = x[n] + x[n+2], flat, both operands 4B-aligned -> 2x
                ht = hpool.tile([P, t_c_max * L], BF16, tag="h", name="h")
                nc.vector.tensor_tensor(
                    ht[:, 0 : N - 2], xt[:, c0 : c0 + N - 2],
                    xt[:, c0 + 2 : c0 + N], mybir.AluOpType.add,
                )
                # s(i,j) = h(i,j) + x(i+1,j+1) over the valid 14x6
                s = spool.tile([P, t_c_max * OUT], BF16, tag="s", name="s")[:, : t_c * OUT]
                h4 = ht.rearrange("p (t i j) -> p t i j", i=16, j=8)
                x4 = X4[:, off : off + t_c]
                s4 = s.rearrange("p (t i j) -> p t i j", i=14, j=6)
                nc.vector.tensor_tensor(
                    s4[:], h4[:, 0:t_c, 0:14, 0:6], x4[:, :, 1:15, 1:7],
                    mybir.AluOpType.add,
                )
                # s += h(i+2,j): both operands contiguous-run 4B-aligned
                nc.vector.tensor_tensor(
                    s4[:], s4[:], h4[:, 0:t_c, 2:16, 0:6], mybir.AluOpType.add,
                )
                # relu in place (bf16) on ACT; f32 cast happens in the store
                nc.scalar.activation(
                    s[:], s[:], mybir.ActivationFunctionType.Relu
                )
                pending.append((yf[:, off * OUT : (off + t_c) * OUT], s[:]))
                off += t_c
            # stores ride the same SWDGE queue as the loads: the queue
            # FIFO gives loads strict HBM priority, stores burst after.
            for dst, src in pending:
                nc.gpsimd.dma_start(out=dst, in_=src)  # bf16 -> f32 cast

    if split_waits:
        split_multi_waits(nc)
    return nc


_nc_cache = {}


def _get_program():
    if "nc" not in _nc_cache:
        _nc_cache["nc"] = build_program()
    return _nc_cache["nc"]


def kernel(x):
    x = np.ascontiguousarray(np.asarray(x, dtype=np.float32))
    assert x.shape == (B, W, L), x.shape

    nc = _get_program()
    shards = x.reshape(N_CORES, ROWS_PER_CORE, L)
    in_maps = [{"x": shards[i]} for i in range(N_CORES)]
    res = run_bass_kernel_spmd(nc, in_maps, core_ids=list(range(N_CORES)))
    out = np.concatenate(
        [res.results[i]["y"].reshape(-1, W, OUT) for i in range(N_CORES)], axis=0
    )
    return out
